# revision 16
# baseline (speedup 1.0000x reference)
"""Trainium2 Bass kernel for an InteractionPPBlock-style GNN message-passing layer.

Strategy (8 NeuronCores):
  * Edges partitioned 25000/core by idx_ji ownership; triplets land on the core
    that owns their scatter destination, so segment_sum is local (one-hot
    matmul into PSUM).
  * Host-side graph partitioning: each core's edges are dealt into 196 bins of
    128 slots with degree-balanced bin sums (snake deal + swap fixups) so every
    bin holds exactly K*128 triplets after padding -> uniform SPMD program.
  * Phase A (sharded): each core computes the gather table
    x_kjd = swish((swish(x@w_kj+b_kj)*rbf_e) @ w_down) for ITS OWN 25088 edges
    only (fp8 output), then an AllGather replicates the full fp8 table to every
    core's DRAM.  Gather is an indirect DMA of 64B rows.
  * Dense per-edge tail (x_ji, w_up, residual blocks) runs in fp16 on
    feature-transposed 1024-wide tiles; weights pre-cast to fp16 on host.
"""

import math
from contextlib import ExitStack

import numpy as np
import ml_dtypes

import concourse.bass as bass
import concourse.mybir as mybir
import concourse.tile as tile
from concourse import bacc
from concourse.bass_utils import run_bass_kernel_spmd

F32 = mybir.dt.float32
F16 = mybir.dt.float16
F8 = mybir.dt.float8e4
I32 = mybir.dt.int32

NP_F8 = ml_dtypes.float8_e4m3fn

HID, INT, BAS, NR, NS = 128, 64, 8, 6, 7
SR = NS * NR  # 42
P = 128

TABLE_F8 = True  # gather table dtype: fp8e4m3 (else fp16)
ACT_FN = mybir.ActivationFunctionType.Silu  # swapped for Sigmoid in sim tests


# ----------------------------------------------------------------------------
# Host-side graph partitioning (free: runs in numpy, not on device)
# ----------------------------------------------------------------------------
def _balance_bins(deg, nblk, cap):
    """Assign len(deg) edges to nblk bins of exactly <=P edges, minimizing the
    max bin degree-sum; returns per-edge bin id."""
    n = len(deg)
    order = np.argsort(-deg, kind="stable")
    r = np.arange(n)
    q, pos = r // nblk, r % nblk
    bins = np.where(q % 2 == 0, pos, nblk - 1 - pos)
    asn = np.empty(n, dtype=np.int64)
    asn[order] = bins
    binsum = np.bincount(asn, weights=deg.astype(np.float64), minlength=nblk)
    for _ in range(2000):
        bmax = int(binsum.argmax())
        if binsum[bmax] <= cap:
            break
        bmin = int(binsum.argmin())
        need = binsum[bmax] - cap
        room = cap - binsum[bmin]
        ii = np.nonzero(asn == bmax)[0]
        jj = np.nonzero(asn == bmin)[0]
        delta = deg[ii][:, None].astype(np.int64) - deg[jj][None, :]
        ok = (delta > 0) & (delta <= room) & (delta >= min(need, room))
        if ok.any():
            a, b = np.argwhere(ok)[0]
        else:
            d2 = np.where(delta <= room, delta, -1)
            a, b = np.unravel_index(np.argmax(d2), d2.shape)
            if d2[a, b] <= 0:
                break
        i, j = ii[a], jj[b]
        asn[i], asn[j] = bmin, bmax
        d = deg[i] - deg[j]
        binsum[bmax] -= d
        binsum[bmin] += d
    return asn, int(binsum.max())


def _preprocess(x, rbf, sbf, idx_kj, idx_ji, n_cores):
    E = x.shape[0]
    T = sbf.shape[0]
    eper = E // n_cores                      # original edges per core
    nblk = math.ceil(eper / P)
    if nblk % 4:
        nblk += 4 - nblk % 4
    eperc = nblk * P                         # padded edges per core
    etot = n_cores * eperc

    idx_kj = idx_kj.astype(np.int64)
    idx_ji = idx_ji.astype(np.int64)
    owner_t = idx_ji // eper                 # triplet -> core
    deg = np.bincount(idx_ji, minlength=E)

    # degree-balanced binning per core (target K*P bin capacity)
    newlocal = np.empty(E, dtype=np.int64)
    binsum_max = 0
    for c in range(n_cores):
        lo, hi = c * eper, (c + 1) * eper
        asn, mx = _balance_bins(deg[lo:hi], nblk, 10 * P)
        binsum_max = max(binsum_max, mx)
        # slot = rank within bin (bins have <=P members by construction)
        o = np.argsort(asn, kind="stable")
        cnt = np.bincount(asn, minlength=nblk)
        starts = np.zeros(nblk, dtype=np.int64)
        starts[1:] = np.cumsum(cnt)[:-1]
        rank = np.empty(eper, dtype=np.int64)
        rank[o] = np.arange(eper) - np.repeat(starts, cnt)
        newlocal[lo:hi] = asn * P + rank

    K = max(1, math.ceil(binsum_max / P))
    cap = K * P
    nchunk = nblk * K
    tpad = nblk * cap
    new_global = (np.arange(E) // eper) * eperc + newlocal

    # table row layout: shard-major [core][partition p][block j][feat]
    nblkA = eperc // P
    eg = np.arange(etot)
    cg, lg = eg // eperc, eg % eperc
    table_row = cg * eperc + (lg % P) * nblkA + lg // P   # table row of edge e

    per_core = []
    for c in range(n_cores):
        tri = np.nonzero(owner_t == c)[0]
        nlji = newlocal[idx_ji[tri]]
        b_of, s_of = nlji // P, nlji % P
        o2 = np.argsort(b_of * P + s_of, kind="stable")
        tri, b_of, s_of = tri[o2], b_of[o2], s_of[o2]
        cnt = np.bincount(b_of, minlength=nblk)
        starts = np.zeros(nblk, dtype=np.int64)
        starts[1:] = np.cumsum(cnt)[:-1]
        rank = np.arange(len(tri)) - np.repeat(starts, cnt)
        pos = b_of * cap + rank

        kj_new = np.zeros(tpad, dtype=np.int32)
        kj_new[pos] = table_row[new_global[idx_kj[tri]]].astype(np.int32)
        ji_sh = np.zeros(tpad, dtype=np.float16)
        ji_sh[pos] = s_of.astype(np.float16)
        sbf_pad = np.zeros((tpad, SR), dtype=np.float16)
        sbf_pad[pos] = sbf[tri].astype(np.float16)

        idx_grid = np.ascontiguousarray(kj_new.reshape(nchunk, P).T)
        ji_grid = ji_sh.reshape(nchunk, P).T.astype(np.int64)  # [P, nchunk]
        ohg = np.zeros((P, nchunk * P), dtype=np.float16)
        tt = np.arange(P)[:, None]
        cols = np.arange(nchunk)[None, :] * P + ji_grid
        ohg[tt, cols] = 1.0
        # sbf^T single-stack [42, nchunk*P] (keeps all PE operands at base
        # partition 0 -- base-64 stationary + sub-bank PSUM dst faults).
        sbf_t = np.ascontiguousarray(
            sbf_pad.reshape(nchunk * P, SR).T)
        per_core.append(dict(idxg=idx_grid, ohg=ohg, sbft=sbf_t))

    # globally renumbered x / rbf
    x_g = np.zeros((etot, HID), dtype=np.float32)
    x_g[new_global] = x
    rbf_g = np.zeros((etot, NR), dtype=np.float32)
    rbf_g[new_global] = rbf

    nsb_a = eperc // 512                     # phase-A chunks per core (49)
    ngrp_r = math.ceil(nsb_a / 3)
    for c in range(n_cores):
        sl = slice(c * eperc, (c + 1) * eperc)
        per_core[c]["xt16c"] = np.ascontiguousarray(
            x_g[sl].T.astype(np.float16))
        # rbf^T 3-stacked at partition offsets {0, 32, 64}
        rbf_c = np.zeros((ngrp_r * 3 * 512, NR), dtype=np.float32)
        rbf_c[:eperc] = rbf_g[sl]
        Rt = (rbf_c.reshape(ngrp_r, 3, 512, NR).transpose(1, 3, 0, 2)
              .reshape(3, NR, ngrp_r * 512).astype(np.float16))
        rbfp = np.zeros((64 + NR, ngrp_r * 512), dtype=np.float16)
        for g in range(3):
            rbfp[32 * g:32 * g + NR] = Rt[g]
        per_core[c]["rbfpc"] = rbfp

    dims = dict(n_cores=n_cores, E=E, T=T, eper=eper, nblk=nblk, eperc=eperc,
                etot=etot, K=K, cap=cap, nchunk=nchunk, nsb_a=nsb_a,
                ngrp_r=ngrp_r, nblkA=nblkA)
    shared = dict()
    return dims, shared, per_core, new_global


# ----------------------------------------------------------------------------
# Device program
# ----------------------------------------------------------------------------
def _build(nc, d):
    nblk, K, nchunk = d["nblk"], d["K"], d["nchunk"]
    eperc, nsb_a, ngrp_r = d["eperc"], d["nsb_a"], d["ngrp_r"]
    etot, nblkA = d["etot"], d["nblkA"]
    n_cores = d["n_cores"]
    nsb_c = nblk // 4
    TDT = F8 if TABLE_F8 else F16

    def din(name, shape, dt):
        return nc.dram_tensor(name, shape, dt, kind="ExternalInput").ap()

    xt16c = din("xt16c", [P, eperc], F16)
    rbfpc = din("rbfpc", [64 + NR, ngrp_r * 512], F16)
    sbft = din("sbft", [SR, nchunk * P], F16)
    idxg = din("idxg", [P, nchunk], I32)
    ohg = din("ohg", [P, nchunk * P], F16)

    # fp16 weights (host-cast); stacked rbf/sbf folded weights
    w16n = ["w_kj16", "w_down16", "w_ji16", "w_up16", "rb0_w116", "rb0_w216",
            "w_lin16", "ra0_w116", "ra0_w216", "ra1_w116", "ra1_w216"]
    wshape = dict(w_down16=[HID, INT], w_up16=[INT, HID])
    Wsb2 = din("Wsb2", [SR, INT], F16)
    Wr3 = din("Wr3", [64 + NR, HID], F16)
    W = {n: din(n, wshape.get(n, [HID, HID]), F16) for n in w16n}
    bn = ["b_kj", "b_ji", "b_lin", "rb0_b1", "rb0_b2", "ra0_b1", "ra0_b2",
          "ra1_b1", "ra1_b2"]
    B = {n: din(n, [P, 1], F32) for n in bn}

    shard = nc.dram_tensor("shard", [P, nblkA * INT], TDT).ap()
    table = nc.dram_tensor("table", [etot, INT], TDT).ap()
    outt = nc.dram_tensor("outt", [P, eperc], F16, kind="ExternalOutput").ap()

    Silu = ACT_FN
    MUL, ADD, EQ = (mybir.AluOpType.mult, mybir.AluOpType.add,
                    mybir.AluOpType.is_equal)

    with tile.TileContext(nc) as tc, ExitStack() as ctx:
        cp = ctx.enter_context(tc.tile_pool(name="const", bufs=1))

        wsb = {}
        for n in w16n:
            t = cp.tile(wshape.get(n, [HID, HID]), F16, tag=n)
            nc.sync.dma_start(out=t[:], in_=W[n][:, :])
            wsb[n] = t
        bsb = {}
        for n in bn:
            t = cp.tile([P, 1], F32, tag=f"b_{n}")
            nc.sync.dma_start(out=t[:], in_=B[n][:, :])
            bsb[n] = t
        idx_sb = cp.tile([P, nchunk], I32, tag="idxg")
        nc.sync.dma_start(out=idx_sb[:], in_=idxg[:, :])
        wsb2 = cp.tile([SR, INT], F16, tag="Wsb2")
        nc.sync.dma_start(out=wsb2[:], in_=Wsb2[:, :])
        wr3 = cp.tile([64 + NR, HID], F16, tag="Wr3")
        nc.sync.dma_start(out=wr3[:], in_=Wr3[:, :])
        # resident x (feature-major fp16), used by phases A and C
        xsb = cp.tile([P, eperc], F16, tag="xsb")
        half = eperc // 2
        nc.sync.dma_start(out=xsb[:, :half], in_=xt16c[:, :half])
        nc.sync.dma_start(out=xsb[:, half:], in_=xt16c[:, half:])

        # ---------------- Phase A: sharded gather-table build ---------------
        tbfull = cp.tile([P, nblkA * INT], TDT, tag="tbfull")
        with tc.tile_pool(name="pa_sb", bufs=3) as pa, \
             tc.tile_pool(name="pa_ps", bufs=2, space="PSUM") as pap, \
             tc.tile_pool(name="pa_io", bufs=2) as pio:
            rt = None
            for s in range(nsb_a):
                if s % 3 == 0:
                    rt = pio.tile([64 + NR, 512], F16, tag="rt")
                    nc.sync.dma_start(
                        out=rt[:],
                        in_=rbfpc[:, (s // 3) * 512:(s // 3 + 1) * 512])
                ps1 = pap.tile([P, 512], F32, tag="ps1", space="PSUM")
                nc.tensor.matmul(ps1[:], wsb["w_kj16"][:],
                                 xsb[:, s * 512:(s + 1) * 512],
                                 start=True, stop=True)
                xkj = pa.tile([P, 512], F16, tag="xkj")
                nc.scalar.activation(xkj[:], ps1[:], Silu, bias=bsb["b_kj"][:])
                m = s % 3
                ps2 = pap.tile([P, 512], F32, tag="ps2", space="PSUM")
                nc.tensor.matmul(ps2[:], wr3[32 * m:32 * m + NR, :],
                                 rt[32 * m:32 * m + NR, :],
                                 start=True, stop=True)
                xkm = pa.tile([P, 512], F16, tag="xkm")
                nc.vector.tensor_tensor(out=xkm[:], in0=xkj[:], in1=ps2[:],
                                        op=MUL)
                pd = pap.tile([P, 4 * INT], F32, tag="pd", space="PSUM")
                for j in range(4):
                    nc.tensor.matmul(pd[:, j * INT:(j + 1) * INT],
                                     xkm[:, j * P:(j + 1) * P],
                                     wsb["w_down16"][:],
                                     start=True, stop=True)
                nc.scalar.activation(
                    tbfull[:, s * 4 * INT:(s + 1) * 4 * INT], pd[:], Silu)
            nc.sync.dma_start(out=shard[:, :], in_=tbfull[:])

        # ---------------- AllGather: replicate table shards ------------------
        nc.gpsimd.collective_compute(
            "AllGather", mybir.AluOpType.bypass,
            replica_groups=[list(range(n_cores))],
            ins=[shard[:, :].opt()], outs=[table[:, :].opt()])
        # CC cores run collectives in order; a barrier AllReduce whose
        # output is read back makes the table delivery observable.
        bar_in = nc.dram_tensor("bar_in", [P, 4], F32).ap()
        bar_out = nc.dram_tensor("bar_out", [P, 4], F32).ap()
        bar_sb = cp.tile([P, 4], F32, tag="bar_sb")
        nc.gpsimd.memset(bar_sb[:], 0.0)
        nc.gpsimd.dma_start(out=bar_in[:, :], in_=bar_sb[:])
        nc.gpsimd.collective_compute(
            "AllReduce", mybir.AluOpType.add,
            replica_groups=[list(range(n_cores))],
            ins=[bar_in[:, :].opt()], outs=[bar_out[:, :].opt()])
        bar_rd = cp.tile([P, 4], F32, tag="bar_rd")
        nc.gpsimd.dma_start(out=bar_rd[:], in_=bar_out[:, :])

        # ---------------- Phase B + C: gather/scatter + dense tail ----------
        with tc.tile_pool(name="pb_sb", bufs=3) as pb, \
             tc.tile_pool(name="pb_big", bufs=2) as pbig, \
             tc.tile_pool(name="pb_ps", bufs=2, space="PSUM") as pbp, \
             tc.tile_pool(name="pc_sb", bufs=2) as pc, \
             tc.tile_pool(name="pc_ps", bufs=2, space="PSUM") as pcp:
            csb = 4 * K  # chunks per superblock
            ngr = math.ceil(K / 4)
            aggs = None

            def mm(ps, w, rhs, W):
                # matmul moving-dim/PSUM-bank limit: emit in 512-col pieces
                for o in range(0, W, 512):
                    nc.tensor.matmul(ps[:, o:o + 512], w,
                                     rhs[:, o:o + 512], start=True, stop=True)

            def tail(cols, W):
                """dense per-edge tail over W edges (feature-major fp16)."""
                xl = xsb[:, cols]
                pup = pcp.tile([P, 1024], F32, tag="psC", space="PSUM")
                mm(pup, wsb["w_up16"][:], aggs, W)
                xup = pc.tile([P, 1024], F16, tag="xup")
                nc.scalar.activation(xup[:, :W], pup[:, :W], Silu)
                pji = pcp.tile([P, 1024], F32, tag="psC", space="PSUM")
                mm(pji, wsb["w_ji16"][:], xl, W)
                hji = pc.tile([P, 1024], F16, tag="hji")
                nc.scalar.activation(hji[:, :W], pji[:, :W], Silu,
                                     bias=bsb["b_ji"][:])
                h = pc.tile([P, 1024], F16, tag="h0")
                nc.vector.tensor_tensor(out=h[:, :W], in0=hji[:, :W],
                                        in1=xup[:, :W], op=ADD)

                def res(hin, w1, b1, w2, b2, tg):
                    p1 = pcp.tile([P, 1024], F32, tag="psC", space="PSUM")
                    mm(p1, wsb[w1][:], hin, W)
                    t1 = pc.tile([P, 1024], F16, tag=f"t1{tg}")
                    nc.scalar.activation(t1[:, :W], p1[:, :W], Silu,
                                         bias=bsb[b1][:])
                    p2 = pcp.tile([P, 1024], F32, tag="psC", space="PSUM")
                    mm(p2, wsb[w2][:], t1, W)
                    t2 = pc.tile([P, 1024], F16, tag=f"t2{tg}")
                    nc.scalar.activation(t2[:, :W], p2[:, :W], Silu,
                                         bias=bsb[b2][:])
                    ho = pc.tile([P, 1024], F16, tag=f"h{tg}")
                    nc.vector.tensor_tensor(out=ho[:, :W], in0=hin[:, :W],
                                            in1=t2[:, :W], op=ADD)
                    return ho

                h = res(h, "rb0_w116", "rb0_b1", "rb0_w216", "rb0_b2", "r0")
                pl = pcp.tile([P, 1024], F32, tag="psC", space="PSUM")
                mm(pl, wsb["w_lin16"][:], h, W)
                hl = pc.tile([P, 1024], F16, tag="hl")
                nc.scalar.activation(hl[:, :W], pl[:, :W], Silu,
                                     bias=bsb["b_lin"][:])
                h = pc.tile([P, 1024], F16, tag="h1")
                nc.vector.tensor_tensor(out=h[:, :W], in0=hl[:, :W], in1=xl,
                                        op=ADD)
                h = res(h, "ra0_w116", "ra0_b1", "ra0_w216", "ra0_b2", "a0")
                h = res(h, "ra1_w116", "ra1_b1", "ra1_w216", "ra1_b2", "a1")
                nc.sync.dma_start(out=outt[:, cols], in_=h[:, :W])

            for s in range(nsb_c):
                gt = pbig.tile([P, csb * INT], TDT, tag="gt")
                nc.gpsimd.indirect_dma_start(
                    out=gt[:], out_offset=None, in_=table[:, :],
                    in_offset=bass.IndirectOffsetOnAxis(
                        ap=idx_sb[:, s * csb:(s + 1) * csb], axis=0))
                st = pbig.tile([SR, csb * P], F16, tag="st")
                nc.sync.dma_start(out=st[:],
                                  in_=sbft[:, s * csb * P:(s + 1) * csb * P])
                ohs = pbig.tile([P, csb * P], F16, tag="ohs")
                nc.sync.dma_start(out=ohs[:],
                                  in_=ohg[:, s * csb * P:(s + 1) * csb * P])
                if s % 2 == 0:
                    aggs = pc.tile([INT, 1024], F16, tag="aggs")
                for j in range(4):
                    pagg = pbp.tile([INT, P], F32, tag="pagg", space="PSUM")
                    for kg in range(ngr):
                        g = min(4, K - 4 * kg)
                        k0 = 4 * kg
                        psE = pbp.tile([P, 4 * INT], F32, tag="psE",
                                       space="PSUM")
                        for q in range(g):
                            cc = j * K + k0 + q
                            nc.tensor.matmul(
                                psE[:, q * INT:(q + 1) * INT],
                                st[:, cc * P:(cc + 1) * P],
                                wsb2[:],
                                start=True, stop=True)
                        msg4 = pb.tile([P, 4 * INT], F16, tag="msg")
                        gt0 = (j * K + k0) * INT
                        nc.vector.tensor_tensor(
                            out=msg4[:, :g * INT],
                            in0=gt[:, gt0:gt0 + g * INT],
                            in1=psE[:, :g * INT], op=MUL)
                        for q in range(g):
                            cc = j * K + k0 + q
                            nc.tensor.matmul(
                                pagg[:], msg4[:, q * INT:(q + 1) * INT],
                                ohs[:, cc * P:(cc + 1) * P],
                                start=(kg == 0 and q == 0),
                                stop=(k0 + q == K - 1))
                    nc.vector.tensor_copy(
                        out=aggs[:, (s % 2) * 512 + j * P:
                                 (s % 2) * 512 + (j + 1) * P],
                        in_=pagg[:])
                if s % 2 == 1:
                    tail(slice((s - 1) * 512, (s + 1) * 512), 1024)
                elif s == nsb_c - 1:
                    tail(slice(s * 512, (s + 1) * 512), 512)
    return outt


# ----------------------------------------------------------------------------
def _run(inputs, n_cores=8, trace=False):
    x = np.asarray(inputs["x"], np.float32)
    rbf = np.asarray(inputs["rbf"], np.float32)
    sbf = np.asarray(inputs["sbf"], np.float32)
    idx_kj = np.asarray(inputs["idx_kj"])
    idx_ji = np.asarray(inputs["idx_ji"])

    d, shared, per_core, new_global = _preprocess(
        x, rbf, sbf, idx_kj, idx_ji, n_cores)

    nc = bacc.Bacc("TRN2", target_bir_lowering=False, debug=False,
                   enable_asserts=False, num_devices=n_cores)
    _build(nc, d)
    nc.compile()

    f32, f16 = np.float32, np.float16
    base = dict(shared)
    wmap = dict(w_kj16="w_kj", w_down16="w_down", w_ji16="w_ji",
                w_up16="w_up", rb0_w116="rb0_w1", rb0_w216="rb0_w2",
                w_lin16="w_lin", ra0_w116="ra0_w1", ra0_w216="ra0_w2",
                ra1_w116="ra1_w1", ra1_w216="ra1_w2")
    for k16, k in wmap.items():
        base[k16] = np.ascontiguousarray(np.asarray(inputs[k], f32), f32
                                         ).astype(f16)
    wsb_f = (np.asarray(inputs["w_sbf1"], f32) @
             np.asarray(inputs["w_sbf2"], f32)).astype(f16)
    base["Wsb2"] = np.ascontiguousarray(wsb_f)
    wr_f = (np.asarray(inputs["w_rbf1"], f32) @
            np.asarray(inputs["w_rbf2"], f32)).astype(f16)
    Wr3 = np.zeros((64 + NR, HID), dtype=f16)
    for g in range(3):
        Wr3[32 * g:32 * g + NR] = wr_f
    base["Wr3"] = Wr3
    for bnm in ["b_kj", "b_ji", "b_lin", "rb0_b1", "rb0_b2", "ra0_b1",
                "ra0_b2", "ra1_b1", "ra1_b2"]:
        base[bnm] = np.ascontiguousarray(
            np.asarray(inputs[bnm], f32).reshape(P, 1))

    in_maps = []
    for c in range(n_cores):
        m = dict(base)
        m.update(per_core[c])
        in_maps.append(m)

    res = run_bass_kernel_spmd(nc, in_maps, core_ids=list(range(n_cores)),
                               trace=trace)
    h_full = np.concatenate([res.results[c]["outt"].T for c in range(n_cores)],
                            axis=0)
    out = h_full[new_global].astype(np.float32)
    return out, res


def kernel(**inputs):
    out, _ = _run(inputs, n_cores=8, trace=False)
    return out
